# revision 2
# baseline (speedup 1.0000x reference)
"""Trainium2 Bass kernel for nn_AttentiveEncoderPOS — fp8 DoubleRow attention.

Same structure as kernel_v2 (sequence-parallel linear + AllGather + attention)
but the attention matmuls (scores and A@V) run in fp8e4 DoubleRow perf mode
(2 contraction rows per cycle = 2x TensorE throughput):
  - L is computed scaled by 64 (W.T and b pre-scaled host-side) so fp8e4
    quantization of L keeps ~6% relative error instead of hitting subnormals.
  - scores' = L'L'.T = 4096 * scores -> exp scale folds in 1/4096.
  - V' = 64V -> final reciprocal folds in the 1/64.
  - exp() emits fp8 E tiles packed in PAIRS of key tiles [128, 2, 1024] so
    A@V DoubleRow contracts two key tiles per instruction; K.T tiles are
    packed [128, 8ht, 128] so scores DoubleRow contracts two h-tiles.
  - AllGather payload is fp8: 2MB per rank.
"""

import numpy as np

import concourse.bass as bass
import concourse.mybir as mybir
from concourse import bacc
from concourse.tile import TileContext
from concourse.bass_utils import run_bass_kernel_spmd
from concourse.masks import make_identity

N = 8192
H = 1024
VOCAB = 50257
POS = 64
NCORES = 8
NL = N // NCORES          # 1024 rows (queries) per core
P = 128
HT = H // P               # 8 h tiles
K2 = 2 * H
KTI = K2 // P             # 16 contraction tiles for the linear
RT = NL // P              # 8 row tiles per core
KT = N // P               # 64 key tiles globally
QTN = NL // P             # 8 query tiles
BLK = 8                   # key tiles per phase-2 block (= one rank's tiles)
NBLK = KT // BLK
LSCALE = 64.0             # fp8 range scaling of L
SCALE = 1.0 / (32.0 * LSCALE * LSCALE)

BF = mybir.dt.bfloat16
F8 = mybir.dt.float8e4
F32 = mybir.dt.float32
I32 = mybir.dt.int32
EXP = mybir.ActivationFunctionType.Exp
RECIP = mybir.ActivationFunctionType.Reciprocal
DR = mybir.MatmulPerfMode.DoubleRow


def build_nc():
    nc = bacc.Bacc(num_devices=NCORES)
    ids = nc.declare_dram_parameter("ids", [RT, P, 1], I32, isOutput=False)
    pids = nc.declare_dram_parameter("pids", [RT, P, 1], I32, isOutput=False)
    emb = nc.declare_dram_parameter("emb", [VOCAB, H], BF, isOutput=False)
    pemb = nc.declare_dram_parameter("pemb", [POS, H], BF, isOutput=False)
    wt = nc.declare_dram_parameter("wt", [K2, H], BF, isOutput=False)  # 64*W.T
    bias = nc.declare_dram_parameter("bias", [HT, P, 1], F32, isOutput=False)
    out = nc.declare_dram_parameter("out", [NL, H], F32, isOutput=True)

    # AllGather bounce buffers (fp8). cc_in[0] = L'_i.T, cc_in[1] = V'_i.
    cc_in = nc.dram_tensor("cc_in", [2, RT, P, NL], F8)
    cc_out = nc.dram_tensor(
        "cc_out", [NCORES, 2, RT, P, NL], F8, addr_space="Shared"
    )
    cs_d = nc.dram_tensor("cs_d", [NL], F32)  # colsum row->col bounce

    with TileContext(nc) as tc:
        with (
            tc.tile_pool(name="const", bufs=1) as const,
            tc.tile_pool(name="ltq", bufs=1) as ltq,
            tc.tile_pool(name="lbf", bufs=HT) as lbf,
            tc.tile_pool(name="vq", bufs=RT) as vq,
        ):
            ident = const.tile([P, P], BF)
            make_identity(nc, ident[:])
            ones = const.tile([P, 1], F8)
            nc.gpsimd.memset(ones[:], 1.0)
            b_sb = const.tile([P, HT], F32)
            nc.sync.dma_start(
                out=b_sb[:].rearrange("p (h u) -> p h u", h=HT),
                in_=bias.rearrange("h p u -> p h u"),
            )

            # Q.T / own keys, fp8, single packed tile [128, (ht q)]
            qpack = ltq.tile([P, HT * NL], F8, tag="qp", name="qpack")
            qp3 = qpack[:].rearrange("p (h q) -> p h q", h=HT)
            lt_bf = []  # bf16 copy for V transposes
            v_sb = []

            # ---------------- Phase 1: local linear ----------------
            with (
                tc.tile_pool(name="wtp", bufs=KTI) as wtp,
                tc.tile_pool(name="idp", bufs=2) as idp,
                tc.tile_pool(name="xbp", bufs=RT) as xbp,
                tc.tile_pool(name="xtp", bufs=KTI) as xtp,
                tc.tile_pool(name="tps", bufs=2, space="PSUM") as tps,
                tc.tile_pool(name="mps", bufs=2, space="PSUM") as mps,
            ):
                wtb = []
                for k in range(KTI):
                    wb = wtp.tile([P, H], BF, tag="wtb")
                    nc.sync.dma_start(out=wb[:], in_=wt[k * P : (k + 1) * P, :])
                    wtb.append(wb)
                idt = idp.tile([P, RT], I32, tag="id")
                nc.sync.dma_start(
                    out=idt[:].rearrange("p (t u) -> p t u", t=RT),
                    in_=ids.rearrange("t p u -> p t u"),
                )
                pidt = idp.tile([P, RT], I32, tag="pid")
                nc.sync.dma_start(
                    out=pidt[:].rearrange("p (t u) -> p t u", t=RT),
                    in_=pids.rearrange("t p u -> p t u"),
                )
                xbs = []
                for rt in range(RT):
                    xb = xbp.tile([P, K2], BF, tag="xb")
                    nc.gpsimd.indirect_dma_start(
                        out=xb[:, 0:H],
                        out_offset=None,
                        in_=emb[:],
                        in_offset=bass.IndirectOffsetOnAxis(
                            ap=idt[:, rt : rt + 1], axis=0
                        ),
                    )
                    nc.gpsimd.indirect_dma_start(
                        out=xb[:, H:K2],
                        out_offset=None,
                        in_=pemb[:],
                        in_offset=bass.IndirectOffsetOnAxis(
                            ap=pidt[:, rt : rt + 1], axis=0
                        ),
                    )
                    xbs.append(xb)
                # X_i.T via PE transposes
                xts = []
                for k in range(KTI):
                    pt = tps.tile([P, NL], BF, tag="tp")
                    for rt in range(RT):
                        nc.tensor.transpose(
                            pt[:, rt * P : (rt + 1) * P],
                            xbs[rt][:, k * P : (k + 1) * P],
                            ident[:],
                        )
                    xt = xtp.tile([P, NL], BF, tag="xt")
                    nc.vector.tensor_copy(out=xt[:], in_=pt[:])
                    xts.append(xt)
                # L'_i.T = 64*(X W.T + b): wt/bias pre-scaled host-side
                for ht in range(HT):
                    pm = mps.tile([P, NL], F32, tag="pm")
                    for half in range(2):
                        sl = slice(half * 512, (half + 1) * 512)
                        for k in range(KTI):
                            nc.tensor.matmul(
                                pm[:, sl],
                                lhsT=wtb[k][:, ht * P : (ht + 1) * P],
                                rhs=xts[k][:, sl],
                                start=(k == 0),
                                stop=(k == KTI - 1),
                            )
                    nc.vector.tensor_add(
                        out=qpack[:, ht * NL : (ht + 1) * NL],
                        in0=pm[:],
                        in1=b_sb[:, ht : ht + 1].to_broadcast([P, NL]),
                    )
                    lb = lbf.tile([P, NL], BF, tag="lb", name="lb")
                    nc.vector.tensor_add(
                        out=lb[:],
                        in0=pm[:],
                        in1=b_sb[:, ht : ht + 1].to_broadcast([P, NL]),
                    )
                    lt_bf.append(lb)
                    nc.sync.dma_start(
                        out=cc_in[0, ht], in_=qpack[:, ht * NL : (ht + 1) * NL]
                    )
                # V'_i tiles (natural layout) via PE transposes of bf16 L'.T
                for rt in range(RT):
                    pv = tps.tile([P, H], BF, tag="pv")
                    for ht in range(HT):
                        nc.tensor.transpose(
                            pv[:, ht * P : (ht + 1) * P],
                            lt_bf[ht][:, rt * P : (rt + 1) * P],
                            ident[:],
                        )
                    v = vq.tile([P, H], F8, tag="v", name="v")
                    nc.vector.tensor_copy(out=v[:], in_=pv[:])
                    v_sb.append(v)
                    nc.sync.dma_start(out=cc_in[1, rt], in_=v[:])

            nc.gpsimd.collective_compute(
                "AllGather",
                mybir.AluOpType.bypass,
                replica_groups=[list(range(NCORES))],
                ins=[cc_in.ap().opt()],
                outs=[cc_out.ap().opt()],
            )

            # ---------------- Phase 2: attention (fp8 DoubleRow) ----------------
            with (
                tc.tile_pool(name="lkp", bufs=10) as lkp,
                tc.tile_pool(name="ep", bufs=BLK + 2) as ep,
                tc.tile_pool(name="vp2", bufs=BLK + 2) as vp2,
                tc.tile_pool(name="op", bufs=QTN) as op,
                tc.tile_pool(name="fin", bufs=2) as fin,
                tc.tile_pool(name="sps", bufs=2, space="PSUM") as sps,
                tc.tile_pool(name="ops", bufs=2, space="PSUM") as ops,
                tc.tile_pool(name="cps", bufs=1, space="PSUM") as cps,
            ):
                psum_c = [
                    cps.tile([1, 512], F32, tag="cs0", name="psum_c0"),
                    cps.tile([1, 512], F32, tag="cs1", name="psum_c1"),
                ]
                out_sb = [op.tile([P, H], F32, tag="o", name="o") for _ in range(QTN)]
                e2_prev = None
                v2_prev = None
                # software pipeline: scores(blk) emitted before A@V(blk-1)
                for blk in range(NBLK + 1):
                    e2s = []
                    v2s = []
                    if blk < NBLK:
                        e2 = v2t = None
                        for j in range(BLK):
                            kt = blk * BLK + j
                            kk = j % 2
                            if kk == 0:
                                e2 = ep.tile([P, 2 * NL], F8, tag="e2", name="e2")
                                v2t = vp2.tile([P, 2 * H], F8, tag="v2", name="v2t")
                                e2s.append(e2)
                                v2s.append(v2t)
                            ltk = lkp.tile([P, H], F8, tag="lk")
                            nc.sync.dma_start(
                                out=ltk[:].rearrange("p (h c) -> p h c", h=HT),
                                in_=cc_out[
                                    blk, 0, :, :, j * P : (j + 1) * P
                                ].rearrange("h p c -> p h c"),
                            )
                            nc.sync.dma_start(
                                out=v2t[:, kk * H : (kk + 1) * H],
                                in_=cc_out[blk, 1, j],
                            )
                            ltk3 = ltk[:].rearrange("p (h c) -> p h c", h=HT)
                            for qc in range(2):
                                sl = slice(qc * 512, (qc + 1) * 512)
                                ps = sps.tile([P, 512], F32, tag="sp")
                                for hp in range(HT // 2):
                                    nc.tensor.matmul(
                                        ps[:],
                                        lhsT=ltk3[:, 2 * hp : 2 * hp + 2, :],
                                        rhs=qp3[:, 2 * hp : 2 * hp + 2, sl],
                                        start=(hp == 0),
                                        stop=(hp == HT // 2 - 1),
                                        perf_mode=DR,
                                    )
                                nc.scalar.activation(
                                    out=e2[:, kk * NL + qc * 512 : kk * NL + (qc + 1) * 512],
                                    in_=ps[:],
                                    func=EXP,
                                    scale=SCALE,
                                )
                                # softmax denominator (colsum over keys)
                                nc.tensor.matmul(
                                    psum_c[qc][:, :],
                                    lhsT=ones[:, 0:1],
                                    rhs=e2[:, kk * NL + qc * 512 : kk * NL + (qc + 1) * 512],
                                    start=(kt == 0),
                                    stop=(kt == KT - 1),
                                )
                    if blk > 0:
                        for qt in range(QTN):
                            po = ops.tile([P, H], F32, tag="po")
                            for hh in range(2):
                                sl = slice(hh * 512, (hh + 1) * 512)
                                for pr in range(BLK // 2):
                                    e2v = e2_prev[pr][:].rearrange(
                                        "p (k q) -> p k q", k=2
                                    )
                                    v2v = v2_prev[pr][:].rearrange(
                                        "p (k h) -> p k h", k=2
                                    )
                                    nc.tensor.matmul(
                                        po[:, sl],
                                        lhsT=e2v[:, :, qt * P : (qt + 1) * P],
                                        rhs=v2v[:, :, sl],
                                        start=(pr == 0),
                                        stop=(pr == BLK // 2 - 1),
                                        perf_mode=DR,
                                    )
                            if blk == 1:
                                nc.vector.tensor_copy(out=out_sb[qt][:], in_=po[:])
                            else:
                                nc.vector.tensor_add(
                                    out=out_sb[qt][:], in0=out_sb[qt][:], in1=po[:]
                                )
                    e2_prev, v2_prev = e2s, v2s

                # final: out = out_sb / (64 * colsum)
                cs_row = fin.tile([1, NL], F32, tag="csr")
                nc.vector.tensor_copy(out=cs_row[0:1, 0:512], in_=psum_c[0][:])
                nc.vector.tensor_copy(out=cs_row[0:1, 512:NL], in_=psum_c[1][:])
                nc.sync.dma_start(out=cs_d[:], in_=cs_row[0:1, :])
                cs_cols = fin.tile([P, QTN], F32, tag="csc")
                nc.sync.dma_start(
                    out=cs_cols[:],
                    in_=cs_d.rearrange("(q p) -> p q", p=P),
                )
                cs_sc = fin.tile([P, QTN], F32, tag="cssc")
                nc.vector.tensor_scalar_mul(out=cs_sc[:], in0=cs_cols[:], scalar1=LSCALE)
                rec = fin.tile([P, QTN], F32, tag="rec")
                nc.vector.reciprocal(rec[:], cs_sc[:])
                for qt in range(QTN):
                    nc.vector.tensor_mul(
                        out=out_sb[qt][:],
                        in0=out_sb[qt][:],
                        in1=rec[:, qt : qt + 1].to_broadcast([P, H]),
                    )
                    nc.sync.dma_start(
                        out=out[qt * P : (qt + 1) * P, :], in_=out_sb[qt][:]
                    )
    nc.finalize()
    return nc


def _prep_inputs(inputs):
    bf = mybir.dt.np(BF)
    ids = np.asarray(inputs["input_ids"]).astype(np.int32)
    pids = np.asarray(inputs["pos_ids"]).astype(np.int32)
    emb = np.asarray(inputs["emb"], dtype=np.float32).astype(bf)
    pemb = np.asarray(inputs["pos_emb"], dtype=np.float32).astype(bf)
    W = np.asarray(inputs["W"], dtype=np.float32)
    wt = np.ascontiguousarray(W.T * LSCALE).astype(bf)         # [2H, H] 64*W.T
    b = np.asarray(inputs["b"], dtype=np.float32) * LSCALE
    bias = np.ascontiguousarray(b.reshape(HT, P, 1))
    in_maps = []
    for i in range(NCORES):
        in_maps.append(
            {
                "ids": np.ascontiguousarray(ids[i * NL : (i + 1) * NL].reshape(RT, P, 1)),
                "pids": np.ascontiguousarray(
                    pids[i * NL : (i + 1) * NL].reshape(RT, P, 1)
                ),
                "emb": emb,
                "pemb": pemb,
                "wt": wt,
                "bias": bias,
            }
        )
    return in_maps


def run(inputs, trace=False):
    nc = build_nc()
    in_maps = _prep_inputs(inputs)
    res = run_bass_kernel_spmd(nc, in_maps, list(range(NCORES)), trace=trace)
    out = np.concatenate([res.results[i]["out"] for i in range(NCORES)], axis=0)
    return out, res


def kernel(**inputs):
    out, _ = run(inputs, trace=False)
    return out


# revision 4
# speedup vs baseline: 1.1098x; 1.1098x over previous
"""Trainium2 Bass kernel for nn_AttentiveEncoderPOS — fp8 DR + AG-overlapped.

vs kernel_v4:
  - ONE combined AllGather (fp8, 2MB/rank) instead of two (two collectives
    paid two ncfw floors and serialized; one is strictly faster).
  - Phase 2 processes the core's OWN key block first, entirely from SBUF
    (qpack/vpack), overlapping the AllGather; the 7 remote blocks are
    fetched with pid-dependent indirect gathers (rank = (pid+b) % 8), one
    [128x8]-offset indirect DMA per rank per tensor, landing in the same
    [p, (tile, col)] layout as the local packs.
  - A@V(b) emitted right after scores(b); colsum spans all 64 tiles.
"""

import numpy as np

import concourse.bass as bass
import concourse.mybir as mybir
from concourse import bacc
from concourse.tile import TileContext
from concourse.bass_utils import run_bass_kernel_spmd
from concourse.masks import make_identity

N = 8192
H = 1024
VOCAB = 50257
POS = 64
NCORES = 8
NL = N // NCORES          # 1024 rows (queries) per core
P = 128
HT = H // P               # 8 h tiles
K2 = 2 * H
KTI = K2 // P             # 16 contraction tiles for the linear
RT = NL // P              # 8 row tiles per core
KT = N // P               # 64 key tiles globally
QTN = NL // P             # 8 query tiles
BLK = 8                   # key tiles per phase-2 block (= one rank)
ESC = 64.0                # emb scale
WSC = 16.0                # W scale
LSCALE = ESC * WSC        # L' = 1024 * L
SCALE = 1.0 / (32.0 * LSCALE * LSCALE)

BF = mybir.dt.bfloat16
F8 = mybir.dt.float8e4
F32 = mybir.dt.float32
I32 = mybir.dt.int32
U32 = mybir.dt.uint32
EXP = mybir.ActivationFunctionType.Exp
DR = mybir.MatmulPerfMode.DoubleRow
ADD = mybir.AluOpType.add
MULT = mybir.AluOpType.mult
IS_GE = mybir.AluOpType.is_ge


def build_nc():
    nc = bacc.Bacc(num_devices=NCORES)
    ids = nc.declare_dram_parameter("ids", [RT, P, 1], I32, isOutput=False)
    pids = nc.declare_dram_parameter("pids", [RT, P, 1], I32, isOutput=False)
    emb = nc.declare_dram_parameter("emb", [VOCAB, H], BF, isOutput=False)
    pemb = nc.declare_dram_parameter("pemb", [POS, H], BF, isOutput=False)
    wt = nc.declare_dram_parameter("wt", [KTI, P, H], F8, isOutput=False)
    bias = nc.declare_dram_parameter("bias", [HT, P, 1], F32, isOutput=False)
    out = nc.declare_dram_parameter("out", [NL, H], F32, isOutput=True)

    # combined AllGather bounce, per-partition-major: row p holds that
    # lane's L'.T (ht, c) block then its V' (t, h) block, 16KB contiguous.
    CCW = 2 * HT * NL  # 16384
    cc_in = nc.dram_tensor("cc_in", [P, CCW], F8)
    cc_out = nc.dram_tensor("cc_out", [NCORES, P, CCW], F8, addr_space="Shared")
    cc_flat = cc_out.rearrange("r p c -> (r p) c")  # [1024, 16384]
    cs_d = nc.dram_tensor("cs_d", [NL], F32)  # colsum row->col bounce

    with TileContext(nc) as tc:
        with (
            tc.tile_pool(name="const", bufs=1) as const,
            tc.tile_pool(name="ltq", bufs=1) as ltq,
            tc.tile_pool(name="vpk", bufs=1) as vpk,
            tc.tile_pool(name="idxp", bufs=1) as idxp,
        ):
            ident = const.tile([P, P], BF)
            make_identity(nc, ident[:])
            ones2 = const.tile([P, 32], F8)  # k-stride 16B for DR lhsT
            nc.gpsimd.memset(ones2[:], 1.0)
            b_sb = const.tile([P, HT], F32)
            nc.sync.dma_start(
                out=b_sb[:].rearrange("p (h u) -> p h u", h=HT),
                in_=bias.rearrange("h p u -> p h u"),
            )

            # Q.T / own keys and own V, fp8, packed [128, (tile col)]
            qpack = ltq.tile([P, HT * NL], F8, tag="qp", name="qpack")
            qp3 = qpack[:].rearrange("p (h q) -> p h q", h=HT)
            vpack = vpk.tile([P, RT * H], F8, tag="vp", name="vpack")
            vp3 = vpack[:].rearrange("p (t h) -> p t h", t=RT)

            # ---- gather row-indices for the 7 remote ranks ----
            # idx[b][p] = ((pid+1+b)%8)*128 + p
            iota1 = idxp.tile([P, 1], I32, tag="iota1")
            nc.gpsimd.iota(iota1[:], pattern=[[0, 1]], base=0, channel_multiplier=1)
            iota1f = idxp.tile([P, 1], F32, tag="iota1f")
            nc.vector.tensor_copy(out=iota1f[:], in_=iota1[:])
            pid_u = idxp.tile([P, 1], U32, tag="pidu")
            nc.sync.dma_start(
                out=pid_u[:], in_=nc.partition_id_tensor.ap().to_broadcast([P, 1])
            )
            pid_f = idxp.tile([P, 1], F32, tag="pidf")
            nc.vector.tensor_copy(out=pid_f[:], in_=pid_u[:])
            idx_g = []
            for b in range(1, NCORES):
                rb = idxp.tile([P, 1], F32, tag="rb", name="rb")
                # rb = pid + b ; rb -= 8*(rb >= 8) ; rb = rb*128 + p
                nc.vector.tensor_scalar(
                    out=rb[:], in0=pid_f[:], scalar1=float(b), scalar2=None, op0=ADD
                )
                ge = idxp.tile([P, 1], F32, tag="ge", name="ge")
                nc.vector.tensor_scalar(
                    out=ge[:], in0=rb[:], scalar1=8.0, scalar2=-8.0 * 128.0,
                    op0=IS_GE, op1=MULT,
                )
                nc.vector.tensor_scalar(
                    out=rb[:], in0=rb[:], scalar1=128.0, scalar2=None, op0=MULT
                )
                nc.vector.tensor_tensor(out=rb[:], in0=rb[:], in1=ge[:], op=ADD)
                nc.vector.tensor_tensor(out=rb[:], in0=rb[:], in1=iota1f[:], op=ADD)
                il = idxp.tile([P, 1], I32, tag="il", name="il")
                nc.vector.tensor_copy(out=il[:], in_=rb[:])
                idx_g.append(il)

            # ---------------- Phase 1: local linear (fp8 DR) ----------------
            with (
                tc.tile_pool(name="wxp", bufs=2) as wxp,
                tc.tile_pool(name="idp", bufs=2) as idp,
                tc.tile_pool(name="xbp", bufs=RT) as xbp,
                tc.tile_pool(name="lbf", bufs=HT) as lbf,
                tc.tile_pool(name="tps", bufs=2, space="PSUM") as tps,
                tc.tile_pool(name="mps", bufs=2, space="PSUM") as mps,
            ):
                idt = idp.tile([P, RT], I32, tag="id")
                nc.sync.dma_start(
                    out=idt[:].rearrange("p (t u) -> p t u", t=RT),
                    in_=ids.rearrange("t p u -> p t u"),
                )
                pidt = idp.tile([P, RT], I32, tag="pid")
                nc.sync.dma_start(
                    out=pidt[:].rearrange("p (t u) -> p t u", t=RT),
                    in_=pids.rearrange("t p u -> p t u"),
                )
                xbs = []
                for rt in range(RT):
                    xb = xbp.tile([P, K2], BF, tag="xb")
                    nc.gpsimd.indirect_dma_start(
                        out=xb[:, 0:H],
                        out_offset=None,
                        in_=emb[:],
                        in_offset=bass.IndirectOffsetOnAxis(
                            ap=idt[:, rt : rt + 1], axis=0
                        ),
                    )
                    xbs.append(xb)
                for rt in range(RT):
                    nc.gpsimd.indirect_dma_start(
                        out=xbs[rt][:, H:K2],
                        out_offset=None,
                        in_=pemb[:],
                        in_offset=bass.IndirectOffsetOnAxis(
                            ap=pidt[:, rt : rt + 1], axis=0
                        ),
                    )
                wpack = wxp.tile([P, KTI * H], F8, tag="wp", name="wpack")
                nc.sync.dma_start(
                    out=wpack[:].rearrange("p (k h) -> p k h", k=KTI),
                    in_=wt.rearrange("k p h -> p k h"),
                )
                wp3 = wpack[:].rearrange("p (k h) -> p k h", k=KTI)
                xpack = wxp.tile([P, KTI * NL], F8, tag="xp", name="xpack")
                xp3 = xpack[:].rearrange("p (k q) -> p k q", k=KTI)
                for k in range(KTI):
                    pt = tps.tile([P, NL], BF, tag="tp")
                    for rt in range(RT):
                        nc.tensor.transpose(
                            pt[:, rt * P : (rt + 1) * P],
                            xbs[rt][:, k * P : (k + 1) * P],
                            ident[:],
                        )
                    nc.vector.tensor_copy(
                        out=xpack[:, k * NL : (k + 1) * NL], in_=pt[:]
                    )
                lt_bf = []
                for ht in range(HT):
                    pm = mps.tile([P, NL], F32, tag="pm")
                    for half in range(2):
                        sl = slice(half * 512, (half + 1) * 512)
                        for kp in range(KTI // 2):
                            nc.tensor.matmul(
                                pm[:, sl],
                                lhsT=wp3[:, 2 * kp : 2 * kp + 2, ht * P : (ht + 1) * P],
                                rhs=xp3[:, 2 * kp : 2 * kp + 2, sl],
                                start=(kp == 0),
                                stop=(kp == KTI // 2 - 1),
                                perf_mode=DR,
                            )
                    nc.vector.tensor_add(
                        out=qpack[:, ht * NL : (ht + 1) * NL],
                        in0=pm[:],
                        in1=b_sb[:, ht : ht + 1].to_broadcast([P, NL]),
                    )
                    lb = lbf.tile([P, NL], BF, tag="lb", name="lb")
                    nc.vector.tensor_add(
                        out=lb[:],
                        in0=pm[:],
                        in1=b_sb[:, ht : ht + 1].to_broadcast([P, NL]),
                    )
                    lt_bf.append(lb)
                    nc.sync.dma_start(
                        out=cc_in[:, ht * NL : (ht + 1) * NL],
                        in_=qpack[:, ht * NL : (ht + 1) * NL],
                    )
                for rt in range(RT):
                    pv = tps.tile([P, H], BF, tag="pv")
                    for ht in range(HT):
                        nc.tensor.transpose(
                            pv[:, ht * P : (ht + 1) * P],
                            lt_bf[ht][:, rt * P : (rt + 1) * P],
                            ident[:],
                        )
                    nc.vector.tensor_copy(
                        out=vpack[:, rt * H : (rt + 1) * H], in_=pv[:]
                    )
                    nc.sync.dma_start(
                        out=cc_in[:, HT * NL + rt * H : HT * NL + (rt + 1) * H],
                        in_=vpack[:, rt * H : (rt + 1) * H],
                    )

            nc.gpsimd.collective_compute(
                "AllGather",
                mybir.AluOpType.bypass,
                replica_groups=[list(range(NCORES))],
                ins=[cc_in.ap().opt()],
                outs=[cc_out.ap().opt()],
            )

            # ---------------- Phase 2: attention (fp8 DR) ----------------
            with (
                tc.tile_pool(name="ltg", bufs=3) as ltgp,
                tc.tile_pool(name="ep", bufs=BLK // 2 + 2) as ep,
                tc.tile_pool(name="op", bufs=QTN) as op,
                tc.tile_pool(name="fin", bufs=2) as fin,
                tc.tile_pool(name="sps", bufs=2, space="PSUM") as sps,
                tc.tile_pool(name="ops", bufs=2, space="PSUM") as ops,
                tc.tile_pool(name="cps", bufs=1, space="PSUM") as cps,
            ):
                psum_c = [
                    cps.tile([1, 512], F32, tag="cs0", name="psum_c0"),
                    cps.tile([1, 512], F32, tag="cs1", name="psum_c1"),
                ]
                out_sb = [op.tile([P, H], F32, tag="o", name="o") for _ in range(QTN)]
                rec = None

                # prefetch remote gathers (they queue behind the AG on gpsimd)
                lt_srcs = [qp3]
                v_srcs = [vp3]
                for b in range(1, NCORES):
                    g = ltgp.tile([P, CCW], F8, tag="ltg", name="g")
                    nc.gpsimd.indirect_dma_start(
                        out=g[:],
                        out_offset=None,
                        in_=cc_flat[:],
                        in_offset=bass.IndirectOffsetOnAxis(
                            ap=idx_g[b - 1][:, 0:1], axis=0
                        ),
                    )
                    lt_srcs.append(
                        g[:, 0 : HT * NL].rearrange("p (h q) -> p h q", h=HT)
                    )
                    v_srcs.append(
                        g[:, HT * NL : CCW].rearrange("p (t h) -> p t h", t=RT)
                    )

                for b in range(NCORES):
                    lt3 = lt_srcs[b]
                    v3 = v_srcs[b]
                    e2s = []
                    # scores + exp for this block's 8 key tiles
                    for j in range(BLK):
                        kt = b * BLK + j
                        kk = j % 2
                        if kk == 0:
                            e2 = ep.tile([P, 2 * NL], F8, tag="e2", name="e2")
                            e2s.append(e2)
                        for qc in range(2):
                            sl = slice(qc * 512, (qc + 1) * 512)
                            ps = sps.tile([P, 512], F32, tag="sp")
                            for hp in range(HT // 2):
                                nc.tensor.matmul(
                                    ps[:],
                                    lhsT=lt3[:, 2 * hp : 2 * hp + 2, j * P : (j + 1) * P],
                                    rhs=qp3[:, 2 * hp : 2 * hp + 2, sl],
                                    start=(hp == 0),
                                    stop=(hp == HT // 2 - 1),
                                    perf_mode=DR,
                                )
                            nc.scalar.activation(
                                out=e2[:, kk * NL + qc * 512 : kk * NL + (qc + 1) * 512],
                                in_=ps[:],
                                func=EXP,
                                scale=SCALE,
                            )
                        if kk == 1:
                            e2v = e2[:].rearrange("p (k q) -> p k q", k=2)
                            o2 = ones2[:].rearrange("p (k u) -> p k u", k=2)[:, :, 0:1]
                            for qc in range(2):
                                sl = slice(qc * 512, (qc + 1) * 512)
                                nc.tensor.matmul(
                                    psum_c[qc][:, :],
                                    lhsT=o2,
                                    rhs=e2v[:, :, sl],
                                    start=(kt == 1),
                                    stop=(kt == KT - 1),
                                    perf_mode=DR,
                                )
                    if b == NCORES - 1:
                        # normalization factors: runs while A@V(last) computes
                        cs_row = fin.tile([1, NL], F32, tag="csr")
                        nc.vector.tensor_copy(out=cs_row[0:1, 0:512], in_=psum_c[0][:])
                        nc.vector.tensor_copy(out=cs_row[0:1, 512:NL], in_=psum_c[1][:])
                        nc.sync.dma_start(out=cs_d[:], in_=cs_row[0:1, :])
                        cs_cols = fin.tile([P, QTN], F32, tag="csc")
                        nc.sync.dma_start(
                            out=cs_cols[:], in_=cs_d.rearrange("(q p) -> p q", p=P)
                        )
                        cs_sc = fin.tile([P, QTN], F32, tag="cssc")
                        nc.vector.tensor_scalar_mul(
                            out=cs_sc[:], in0=cs_cols[:], scalar1=LSCALE
                        )
                        rec = fin.tile([P, QTN], F32, tag="rec")
                        nc.vector.reciprocal(rec[:], cs_sc[:])
                    # A@V for this block
                    last = b == NCORES - 1
                    for qt in range(QTN):
                        po = ops.tile([P, H], F32, tag="po")
                        for pr in range(BLK // 2):
                            e2v = e2s[pr][:].rearrange("p (k q) -> p k q", k=2)
                            for hh in range(2):
                                sl = slice(hh * 512, (hh + 1) * 512)
                                nc.tensor.matmul(
                                    po[:, sl],
                                    lhsT=e2v[:, :, qt * P : (qt + 1) * P],
                                    rhs=v3[:, 2 * pr : 2 * pr + 2, sl],
                                    start=(pr == 0),
                                    stop=(pr == BLK // 2 - 1),
                                    perf_mode=DR,
                                )
                        if b == 0:
                            nc.vector.tensor_copy(out=out_sb[qt][:], in_=po[:])
                        else:
                            nc.vector.tensor_add(
                                out=out_sb[qt][:], in0=out_sb[qt][:], in1=po[:]
                            )
                        if last:
                            nc.vector.tensor_mul(
                                out=out_sb[qt][:],
                                in0=out_sb[qt][:],
                                in1=rec[:, qt : qt + 1].to_broadcast([P, H]),
                            )
                            nc.sync.dma_start(
                                out=out[qt * P : (qt + 1) * P, :], in_=out_sb[qt][:]
                            )
    nc.finalize()
    return nc


def _prep_inputs(inputs):
    bf = mybir.dt.np(BF)
    f8 = mybir.dt.np(F8)
    ids = np.asarray(inputs["input_ids"]).astype(np.int32)
    pids = np.asarray(inputs["pos_ids"]).astype(np.int32)
    emb = (np.asarray(inputs["emb"], dtype=np.float32) * ESC).astype(bf)
    pemb = (np.asarray(inputs["pos_emb"], dtype=np.float32) * ESC).astype(bf)
    W = np.asarray(inputs["W"], dtype=np.float32)
    wt = np.ascontiguousarray(W.T * WSC).astype(f8).reshape(KTI, P, H)
    b = np.asarray(inputs["b"], dtype=np.float32) * LSCALE
    bias = np.ascontiguousarray(b.reshape(HT, P, 1))
    in_maps = []
    for i in range(NCORES):
        in_maps.append(
            {
                "ids": np.ascontiguousarray(ids[i * NL : (i + 1) * NL].reshape(RT, P, 1)),
                "pids": np.ascontiguousarray(
                    pids[i * NL : (i + 1) * NL].reshape(RT, P, 1)
                ),
                "emb": emb,
                "pemb": pemb,
                "wt": wt,
                "bias": bias,
            }
        )
    return in_maps


def run(inputs, trace=False):
    nc = build_nc()
    in_maps = _prep_inputs(inputs)
    res = run_bass_kernel_spmd(nc, in_maps, list(range(NCORES)), trace=trace)
    out = np.concatenate([res.results[i]["out"] for i in range(NCORES)], axis=0)
    return out, res


def kernel(**inputs):
    out, _ = run(inputs, trace=False)
    return out
